# revision 12
# baseline (speedup 1.0000x reference)
"""Trainium2 Bass kernel: batched recursive Newton-Euler inverse dynamics
(7-dof serial chain) — data-parallel over 8 NeuronCores.

Per core, the 65536-row shard lives as fp32 planes [128 part, 512 free].
Per-link parameters are baked in as immediate constants. The physics is
emitted through a symbolic layer (Val = a*plane + c) that prunes zeros,
folds scales, and chains every n-term linear combination into n-1 fused
scalar_tensor_tensor ops. Ops are recorded into a tiny IR, dead code is
eliminated, and emission assigns temporaries to SBUF "registers" via
linear-scan liveness (Tile pool slot rotation is strict round-robin, so
naive tmp pools deadlock).
"""

import math
from contextlib import ExitStack

import numpy as np

P = 128
D = 7
N_CORES = 8
BATCH = 524288
SHARD = BATCH // N_CORES      # 65536
FD = SHARD // P               # 512


# ---------------------------------------------------------------------------
# symbolic value: a * plane + c   (plane None -> pure constant)
# ---------------------------------------------------------------------------
class Val:
    __slots__ = ("pl", "a", "c")

    def __init__(self, pl, a=1.0, c=0.0):
        self.pl = pl
        self.a = float(a)
        self.c = float(c)
        if pl is None:
            self.a = 0.0

    @property
    def is_const(self):
        return self.pl is None or self.a == 0.0


def VC(c):
    return Val(None, 0.0, c)


class Builder:
    """Backend-agnostic emitter. Each primitive is exactly one instruction."""

    def __init__(self):
        self.n_2src = 0
        self.n_1src = 0
        self.n_trig = 0
        self._ones = None

    # ---- primitives (backends) ----
    def p_stt(self, in0, scalar, in1, op1, dest=None):
        raise NotImplementedError

    def p_tt(self, in0, in1, op, dest=None):
        raise NotImplementedError

    def p_affine(self, in0, scale, bias, dest=None):
        raise NotImplementedError

    def p_sin(self, in0, scale, bias):
        raise NotImplementedError

    def p_ones(self):
        raise NotImplementedError

    def inp(self, name, j):
        raise NotImplementedError

    def out_ap(self, j):
        raise NotImplementedError

    def f_ap(self, j, i):
        raise NotImplementedError

    def state_ap(self, j, i):
        raise NotImplementedError

    def plane_key(self, pl):
        return id(pl)

    def same_plane(self, a, b):
        return a is b

    # ---- helpers ----
    def ones(self):
        if self._ones is None:
            self._ones = self.p_ones()
        return self._ones

    def sincos(self, j):
        s = Val(self.p_sin(self.inp("q", j), 1.0, 0.0))
        c = Val(self.p_sin(self.inp("q", j), 1.0, math.pi / 2))
        self.n_trig += 2
        return s, c

    def lin(self, vals, coefs, const=0.0, dest=None, exact=False, scale_free=False):
        terms = {}
        c_acc = float(const)
        for v, k in zip(vals, coefs):
            k = float(k)
            if k == 0.0:
                continue
            c_acc += k * v.c
            if v.pl is not None and v.a != 0.0:
                key = self.plane_key(v.pl)
                if key in terms:
                    terms[key][1] += k * v.a
                else:
                    terms[key] = [v.pl, k * v.a]
        tl = [(pl, k) for pl, k in terms.values() if k != 0.0]
        if not tl:
            if dest is not None:
                self.n_1src += 1
                self.p_affine(self.ones(), c_acc, 0.0, dest=dest)
                return Val(dest, 1.0, 0.0)
            return VC(c_acc)
        if c_acc != 0.0:
            tl.append((self.ones(), c_acc))
        if len(tl) == 1:
            pl, k = tl[0]
            if dest is not None:
                self.n_1src += 1
                self.p_affine(pl, k, 0.0, dest=dest)
                return Val(dest, 1.0, 0.0)
            if exact and k != 1.0:
                self.n_1src += 1
                return Val(self.p_affine(pl, k, 0.0), 1.0, 0.0)
            return Val(pl, k, 0.0)
        tl.sort(key=lambda t: abs(t[1]))
        cur_pl, cur_k = tl[0]
        for i in range(1, len(tl)):
            pl_i, k_i = tl[i]
            is_last = i == len(tl) - 1
            use_dest = dest is not None and is_last and (scale_free or k_i == 1.0)
            d = dest if use_dest else None
            self.n_2src += 1
            cur_pl = self.p_stt(cur_pl, cur_k / k_i, pl_i, "add", dest=d)
            cur_k = k_i
        if dest is not None and not self.same_plane(cur_pl, dest):
            self.n_1src += 1
            self.p_affine(cur_pl, cur_k, 0.0, dest=dest)
            return Val(dest, 1.0, 0.0)
        if dest is not None:
            return Val(dest, cur_k if scale_free else 1.0, 0.0)
        if exact and cur_k != 1.0:
            self.n_1src += 1
            return Val(self.p_affine(cur_pl, cur_k, 0.0), 1.0, 0.0)
        return Val(cur_pl, cur_k, 0.0)

    def mov(self, v, dest):
        self.n_1src += 1
        if v.pl is None:
            self.p_affine(self.ones(), v.c, 0.0, dest=dest)
        else:
            self.p_affine(v.pl, v.a, v.c, dest=dest)
        return Val(dest, 1.0, 0.0)

    def mul(self, x, y):
        if x.is_const and y.is_const:
            return VC(x.c * y.c)
        if x.is_const:
            x, y = y, x
        if y.is_const:
            return Val(x.pl, x.a * y.c, x.c * y.c)
        xp, yp = x, y
        if xp.c != 0.0:
            self.n_1src += 1
            xp = Val(self.p_affine(xp.pl, 1.0, xp.c / xp.a), xp.a, 0.0)
        if yp.c != 0.0:
            self.n_1src += 1
            yp = Val(self.p_affine(yp.pl, 1.0, yp.c / yp.a), yp.a, 0.0)
        self.n_2src += 1
        out = self.p_tt(xp.pl, yp.pl, "mult")
        return Val(out, xp.a * yp.a, 0.0)

    def cross(self, u, v):
        out = []
        for i in range(3):
            b, c = (i + 1) % 3, (i + 2) % 3
            m1 = self.mul(u[b], v[c])
            m2 = self.mul(u[c], v[b])
            out.append((m1, m2))
        return out

    def matvec(self, M, v):
        return [self.lin(v, [M[i][0], M[i][1], M[i][2]]) for i in range(3)]

    def givens(self, c, s, k, sgn, w, inverse, dests=None):
        a, b = (k + 1) % 3, (k + 2) % 3
        sg = -sgn if inverse else sgn
        out = [None, None, None]
        if w[a].is_const and w[b].is_const:
            out[a] = self.lin([c, s], [w[a].c, -sg * w[b].c])
            out[b] = self.lin([s, c], [sg * w[a].c, w[b].c])
        else:
            ca = self.mul(c, w[a])
            cb = self.mul(c, w[b])
            sa = self.mul(s, w[a])
            sb = self.mul(s, w[b])
            da = dests[a] if dests else None
            db = dests[b] if dests else None
            out[a] = self.lin([ca, sb], [1.0, -sg], dest=da, scale_free=True)
            out[b] = self.lin([sa, cb], [sg, 1.0], dest=db, scale_free=True)
        out[k] = w[k]
        if dests:
            if dests[a] is not None and (out[a].pl is None
                                         or not self.same_plane(out[a].pl, dests[a])):
                out[a] = self.mov(out[a], dests[a])
            if dests[b] is not None and (out[b].pl is None
                                         or not self.same_plane(out[b].pl, dests[b])):
                out[b] = self.mov(out[b], dests[b])
            if dests[k] is not None and not w[k].is_const:
                out[k] = self.mov(w[k], dests[k])
        return out


# ---------------------------------------------------------------------------
# host-side constants
# ---------------------------------------------------------------------------
def host_consts(rot_fix, trans_fix, joint_axes, mass, com, inertia, damping):
    rot_fix = np.asarray(rot_fix, np.float64)
    trans_fix = np.asarray(trans_fix, np.float64)
    joint_axes = np.asarray(joint_axes, np.float64)
    mass = np.asarray(mass, np.float64)
    com = np.asarray(com, np.float64)
    inertia = np.asarray(inertia, np.float64)
    damping = np.asarray(damping, np.float64)
    C = {}
    C["F"] = [rot_fix[j + 1] for j in range(D)]
    C["p"] = [trans_fix[j + 1] for j in range(D)]
    ax = []
    for j in range(D):
        k = int(np.argmax(np.abs(joint_axes[j])))
        ax.append((k, float(np.sign(joint_axes[j][k]))))
    C["ax"] = ax
    C["m"] = [float(mass[j + 1]) for j in range(D)]
    C["mc"] = [mass[j + 1] * com[j + 1] for j in range(D)]
    Io = []
    for j in range(D):
        cc = com[j + 1]
        cs = np.array([[0, -cc[2], cc[1]], [cc[2], 0, -cc[0]], [-cc[1], cc[0], 0]])
        Io.append(inertia[j + 1] + mass[j + 1] * (cs @ cs.T))
    C["Io"] = Io
    C["damping"] = [float(damping[j]) for j in range(D)]
    C["G"] = 9.81
    return C


# ---------------------------------------------------------------------------
# the physics graph (backend-independent)
# ---------------------------------------------------------------------------
def build_rnea(b: Builder, C):
    Z = VC(0.0)
    vl = [Z, Z, Z]
    va = [Z, Z, Z]
    al = [Z, Z, VC(C["G"])]
    aa = [Z, Z, Z]
    fstore = [[None] * 6 for _ in range(D)]
    for j in range(D):
        F = C["F"][j]
        p = C["p"][j]
        k, sg = C["ax"][j]
        a_, b_ = (k + 1) % 3, (k + 2) % 3
        s, c = b.sincos(j)
        qd = Val(b.inp("qd", j))
        qdd = Val(b.inp("qdd", j))
        Ft = F.T.tolist()

        def dvec(x, y):
            out = []
            for i in range(3):
                bb, cc = (i + 1) % 3, (i + 2) % 3
                out.append(b.lin([x[i], y[cc], y[bb]], [1.0, -p[bb], p[cc]]))
            return out

        u_vl = b.matvec(Ft, dvec(vl, va))
        u_va = b.matvec(Ft, va)
        u_al = b.matvec(Ft, dvec(al, aa))
        u_aa = b.matvec(Ft, aa)
        std = lambda i: b.state_ap(j, i)
        vl_i = b.givens(c, s, k, sg, u_vl, True, dests=[std(0), std(1), std(2)])
        va_r = b.givens(c, s, k, sg, u_va, True,
                        dests=[std(3 + i) if i != k else None for i in range(3)])
        va_i = list(va_r)
        va_i[k] = b.lin([va_r[k], qd], [1.0, sg], dest=std(3 + k), scale_free=True)
        al_r = b.givens(c, s, k, sg, u_al, True,
                        dests=[std(6 + i) if i == k else None for i in range(3)])
        aa_r = b.givens(c, s, k, sg, u_aa, True)
        aa_i = list(aa_r)
        aa_i[k] = b.lin([aa_r[k], qdd], [1.0, sg], dest=std(9 + k), scale_free=True)
        ek = [0.0, 0.0, 0.0]
        ek[k] = 1.0
        al_i = list(al_r)
        for i in (a_, b_):
            bb, cc = (i + 1) % 3, (i + 2) % 3
            cva = b.lin([va_i[bb], va_i[cc]], [ek[cc], -ek[bb]])
            m1 = b.mul(cva, qd)
            aa_i[i] = b.lin([aa_r[i], m1], [1.0, sg], dest=std(9 + i),
                            scale_free=True)
            cvl = b.lin([vl_i[bb], vl_i[cc]], [ek[cc], -ek[bb]])
            m2 = b.mul(cvl, qd)
            al_i[i] = b.lin([al_r[i], m2], [1.0, sg], dest=std(6 + i),
                            scale_free=True)
        vl, va, al, aa = vl_i, va_i, al_i, aa_i

        # ---- force for this joint ----
        m = C["m"][j]
        mc = C["mc"][j].tolist()
        Io = C["Io"][j]
        Iv_l = [b.lin([vl[i], va[(i + 1) % 3], va[(i + 2) % 3]],
                      [m, mc[(i + 2) % 3], -mc[(i + 1) % 3]]) for i in range(3)]
        Ia_l = [b.lin([al[i], aa[(i + 1) % 3], aa[(i + 2) % 3]],
                      [m, mc[(i + 2) % 3], -mc[(i + 1) % 3]]) for i in range(3)]
        Iv_a = [b.lin([va[0], va[1], va[2], vl[(i + 2) % 3], vl[(i + 1) % 3]],
                      [Io[i][0], Io[i][1], Io[i][2],
                       mc[(i + 1) % 3], -mc[(i + 2) % 3]]) for i in range(3)]
        Ia_a = [b.lin([aa[0], aa[1], aa[2], al[(i + 2) % 3], al[(i + 1) % 3]],
                      [Io[i][0], Io[i][1], Io[i][2],
                       mc[(i + 1) % 3], -mc[(i + 2) % 3]]) for i in range(3)]
        cv1 = b.cross(va, Iv_l)
        for i in range(3):
            m1, m2 = cv1[i]
            fstore[j][i] = b.lin([Ia_l[i], m1, m2], [1.0, 1.0, -1.0],
                                 dest=b.f_ap(j, i), scale_free=True)
        cv2 = b.cross(va, Iv_a)
        cv3 = b.cross(vl, Iv_l)
        for i in range(3):
            m1, m2 = cv2[i]
            m3, m4 = cv3[i]
            fstore[j][3 + i] = b.lin([Ia_a[i], m1, m2, m3, m4],
                                     [1.0, 1.0, -1.0, 1.0, -1.0],
                                     dest=b.f_ap(j, 3 + i), scale_free=True)

    # ---- backward pass ----
    cl = [Z, Z, Z]
    ca = [Z, Z, Z]
    for j in range(D - 1, -1, -1):
        F = C["F"][j]
        p = C["p"][j]
        k, sg = C["ax"][j]
        pp = (F.T @ p).tolist()
        s, c = b.sincos(j)
        f_l = fstore[j][:3]
        f_a = fstore[j][3:]
        tl = [b.lin([f_l[i], cl[i]], [1.0, 1.0]) for i in range(3)]
        ta = [b.lin([f_a[i], ca[i]], [1.0, 1.0]) for i in range(3)]
        b.lin([ta[k], Val(b.inp("qd", j))], [sg, C["damping"][j]],
              dest=b.out_ap(j))
        if j == 0:
            continue
        w_l = b.givens(c, s, k, sg, tl, False)
        w_a = b.givens(c, s, k, sg, ta, False)
        x = []
        for i in range(3):
            bb, cc = (i + 1) % 3, (i + 2) % 3
            x.append(b.lin([w_a[i], w_l[cc], w_l[bb]], [1.0, pp[bb], -pp[cc]]))
        cl = b.matvec(F.tolist(), w_l)
        ca = b.matvec(F.tolist(), x)


# ---------------------------------------------------------------------------
# numpy backend (validation)
# ---------------------------------------------------------------------------
class NumpyBuilder(Builder):
    def __init__(self, q, qd, qdd):
        super().__init__()
        self.q, self.qd, self.qdd = q, qd, qdd
        self.N = q.shape[0]
        self.out = np.zeros((self.N, D), np.float32)
        self._f = {}

    def _w(self, r, dest):
        if dest is not None:
            dest[...] = r
            return dest
        return r

    def _f32(self, x):
        return np.asarray(x, np.float32)

    def p_stt(self, in0, scalar, in1, op1, dest=None):
        r = self._f32(in0 * np.float32(scalar))
        if op1 == "add":
            r = self._f32(r + in1)
        elif op1 == "subtract":
            r = self._f32(r - in1)
        else:
            r = self._f32(r * in1)
        return self._w(r, dest)

    def p_tt(self, in0, in1, op, dest=None):
        if op == "mult":
            r = self._f32(in0 * in1)
        elif op == "add":
            r = self._f32(in0 + in1)
        else:
            r = self._f32(in0 - in1)
        return self._w(r, dest)

    def p_affine(self, in0, scale, bias, dest=None):
        return self._w(self._f32(in0 * np.float32(scale) + np.float32(bias)), dest)

    def p_sin(self, in0, scale, bias):
        return self._f32(np.sin(self._f32(in0 * np.float32(scale) + np.float32(bias))))

    def p_ones(self):
        return np.ones(self.N, np.float32)

    def inp(self, name, j):
        return {"q": self.q, "qd": self.qd, "qdd": self.qdd}[name][:, j].astype(
            np.float32
        )

    def out_ap(self, j):
        return self.out[:, j]

    def f_ap(self, j, i):
        key = (j, i)
        if key not in self._f:
            self._f[key] = np.empty(self.N, np.float32)
        return self._f[key]

    def state_ap(self, j, i):
        return np.empty(self.N, np.float32)


def rnea_numpy(q, qd, qdd, rot_fix, trans_fix, joint_axes, mass, com, inertia,
               damping):
    C = host_consts(rot_fix, trans_fix, joint_axes, mass, com, inertia, damping)
    b = NumpyBuilder(q, qd, qdd)
    build_rnea(b, C)
    return b.out


# ---------------------------------------------------------------------------
# IR backend: records ops on integer-token planes
# ---------------------------------------------------------------------------
class IRBuilder(Builder):
    def __init__(self):
        super().__init__()
        self.ops = []   # (kind, out_token, in_tokens, params)
        self._n = 0

    def _tmp(self):
        self._n += 1
        return ("t", self._n)

    def plane_key(self, pl):
        return pl

    def same_plane(self, a, b):
        return a == b

    def p_stt(self, in0, scalar, in1, op1, dest=None):
        out = dest if dest is not None else self._tmp()
        self.ops.append(("stt", out, (in0, in1), (float(scalar), op1)))
        return out

    def p_tt(self, in0, in1, op, dest=None):
        out = dest if dest is not None else self._tmp()
        self.ops.append(("tt", out, (in0, in1), (op,)))
        return out

    def p_affine(self, in0, scale, bias, dest=None):
        out = dest if dest is not None else self._tmp()
        self.ops.append(("affine", out, (in0,), (float(scale), float(bias))))
        return out

    def p_sin(self, in0, scale, bias):
        out = self._tmp()
        self.ops.append(("sin", out, (in0,), (float(scale), float(bias))))
        return out

    def p_ones(self):
        out = ("ones",)
        self.ops.append(("memset", out, (), (1.0,)))
        return out

    def inp(self, name, j):
        return ("in", name, j)

    def out_ap(self, j):
        return ("out", j)

    def f_ap(self, j, i):
        return ("f", j, i)

    def state_ap(self, j, i):
        return self._tmp()


def dce(ops):
    """drop ops whose results are never used (named 'out'/'f' sinks are live;
    'f' only if read)."""
    needed = set()
    keep = [False] * len(ops)
    for idx in range(len(ops) - 1, -1, -1):
        kind, out, ins, params = ops[idx]
        if out[0] == "out" or out in needed:
            keep[idx] = True
            for t in ins:
                needed.add(t)
    return [op for k2, op in zip(keep, ops) if k2]


def ir_stats(ops):
    from collections import Counter

    c = Counter(k for k, *_ in ops)
    # liveness: peak concurrent tmp planes
    last_use = {}
    for idx, (kind, out, ins, params) in enumerate(ops):
        for t in ins:
            if t[0] == "t":
                last_use[t] = idx
    live = set()
    peak = 0
    for idx, (kind, out, ins, params) in enumerate(ops):
        if out[0] == "t":
            live.add(out)
        peak = max(peak, len(live))
        for t in ins:
            if t[0] == "t" and last_use.get(t) == idx:
                live.discard(t)
    return dict(c), peak


def build_ir(C):
    b = IRBuilder()
    build_rnea(b, C)
    ops = dce(b.ops)
    return ops, b


# ---------------------------------------------------------------------------
# bass emission from IR
# ---------------------------------------------------------------------------
def emit_bass(nc, tc, pools, chunks, out_chunk, ops, fd=FD):
    from concourse import mybir

    f32 = mybir.dt.float32
    ALU = {"add": mybir.AluOpType.add, "subtract": mybir.AluOpType.subtract,
           "mult": mybir.AluOpType.mult}

    last_use = {}
    for idx, (kind, out, ins, params) in enumerate(ops):
        for t in ins:
            if t[0] == "t":
                last_use[t] = idx

    ftiles = {}
    tmp_ap = {}         # token -> AP
    reg_of = {}         # token -> reg index
    free_regs = []
    n_regs = 0
    serial = 0

    def named_ap(tok):
        nonlocal serial
        if tok[0] == "in":
            _, name, j = tok
            v = chunks[name].rearrange("p (f d) -> p d f", d=D)
            return v[:, j, :]
        if tok[0] == "out":
            v = out_chunk.rearrange("p (f d) -> p d f", d=D)
            return v[:, tok[1], :]
        if tok[0] == "f":
            _, j, i = tok
            if j not in ftiles:
                serial += 1
                ftiles[j] = pools["fst"].tile([P, 6 * fd], f32, tag=f"f{j}",
                                              name=f"f{j}", bufs=1)
            t = ftiles[j]
            return t[:, i * fd:(i + 1) * fd]
        if tok[0] == "ones":
            return ones_ap
        raise KeyError(tok)

    def get_ap(tok):
        if tok[0] == "t":
            return tmp_ap[tok]
        return named_ap(tok)

    def alloc_out(tok, idx):
        nonlocal n_regs, serial
        if tok[0] != "t":
            return named_ap(tok)
        if free_regs:
            r = free_regs.pop()
        else:
            r = n_regs
            n_regs += 1
        reg_of[tok] = r
        serial += 1
        t = pools["reg"].tile([P, fd], f32, tag=f"r{r}", name=f"v{serial}",
                              bufs=1)
        tmp_ap[tok] = t[:, :]
        return tmp_ap[tok]

    def release_ins(ins, idx):
        for t in ins:
            if t[0] == "t" and last_use.get(t) == idx:
                r = reg_of.pop(t, None)
                if r is not None:
                    free_regs.append(r)

    ones_ap = None
    for idx, (kind, out, ins, params) in enumerate(ops):
        if kind == "memset":
            serial += 1
            t = pools["misc"].tile([P, fd], f32, tag="ones", name="ones", bufs=1)
            ones_ap = t[:, :]
            nc.vector.memset(ones_ap, 1.0)
            continue
        out_ap = alloc_out(out, idx)
        if kind == "stt":
            scalar, op1 = params
            nc.vector.scalar_tensor_tensor(out_ap, get_ap(ins[0]), scalar,
                                           get_ap(ins[1]),
                                           mybir.AluOpType.mult, ALU[op1])
        elif kind == "tt":
            nc.vector.tensor_tensor(out_ap, get_ap(ins[0]), get_ap(ins[1]),
                                    ALU[params[0]])
        elif kind == "affine":
            scale, bias = params
            nc.scalar.activation(out_ap, get_ap(ins[0]),
                                 mybir.ActivationFunctionType.Copy,
                                 bias=bias, scale=scale)
        elif kind == "sin":
            # sin(x + b) with range reduction: the ACT Sin spline is only
            # accurate for |arg| <= pi. r = round((x+b)/2pi) via the fp32
            # magic-constant trick; z = x - 2pi*r; result = Sin(z + b).
            scale, bias = params
            assert scale == 1.0
            src = get_ap(ins[0])
            TWO_PI = 2.0 * math.pi
            MAGIC = 12582912.0  # 1.5 * 2**23
            serial += 1
            scr = pools["misc"].tile([P, fd], f32, tag="trigscratch",
                                     name=f"trs{serial}", bufs=2)
            u = scr[:, :]
            nc.vector.tensor_scalar(u, src, bias, 1.0 / TWO_PI,
                                    mybir.AluOpType.add, mybir.AluOpType.mult)
            nc.vector.tensor_scalar(u, u, MAGIC, MAGIC,
                                    mybir.AluOpType.add,
                                    mybir.AluOpType.subtract)
            nc.vector.scalar_tensor_tensor(out_ap, u, -TWO_PI, src,
                                           mybir.AluOpType.mult,
                                           mybir.AluOpType.add)
            nc.scalar.activation(out_ap, out_ap,
                                 mybir.ActivationFunctionType.Sin,
                                 bias=bias, scale=1.0)
        else:
            raise ValueError(kind)
        release_ins(ins, idx)
    return n_regs


def _build_nc(C, verbose=False):
    import concourse.bacc as bacc
    import concourse.tile as tile_mod
    from concourse import mybir

    ops, bstat = build_ir(C)
    if verbose:
        stats, peak = ir_stats(ops)
        print("IR ops:", stats, "peak live tmps:", peak)

    nc = bacc.Bacc()
    f32 = mybir.dt.float32
    # register pi/2 as a const AP (Sin activation bias must be a const AP)
    halfpi = float(math.pi / 2)
    _ct = nc.alloc_sbuf_tensor("const-f32-halfpi", [128, 1], f32)
    nc.gpsimd.memset(_ct.ap(), halfpi)
    nc.const_aps.aps[(f32, halfpi)] = _ct.ap()
    nc.all_engine_barrier()
    q_d = nc.dram_tensor("q", [SHARD, D], f32, kind="ExternalInput")
    qd_d = nc.dram_tensor("qd", [SHARD, D], f32, kind="ExternalInput")
    qdd_d = nc.dram_tensor("qdd", [SHARD, D], f32, kind="ExternalInput")
    tau_d = nc.dram_tensor("tau", [SHARD, D], f32, kind="ExternalOutput")

    with ExitStack() as ctx:
        tc = ctx.enter_context(tile_mod.TileContext(nc))
        io_pool = ctx.enter_context(tc.tile_pool(name="io", bufs=1))
        fst_pool = ctx.enter_context(tc.tile_pool(name="fst", bufs=1))
        reg_pool = ctx.enter_context(tc.tile_pool(name="reg", bufs=1))
        misc_pool = ctx.enter_context(tc.tile_pool(name="misc", bufs=1))
        pools = {"io": io_pool, "fst": fst_pool, "reg": reg_pool,
                 "misc": misc_pool}

        chunks = {}
        for name, dram in (("q", q_d), ("qd", qd_d), ("qdd", qdd_d)):
            t = io_pool.tile([P, D * FD], f32, tag=f"io_{name}",
                             name=f"ch_{name}", bufs=1)
            nc.sync.dma_start(t[:, :],
                              dram[:, :].rearrange("(p f) d -> p (f d)", p=P))
            chunks[name] = t

        # out chunk shares the qdd slot (qdd is fully consumed by the forward
        # pass before any tau is written)
        out_chunk = io_pool.tile([P, D * FD], f32, tag="io_qdd", name="ch_out",
                                 bufs=1)

        n_regs = emit_bass(nc, tc, pools, chunks, out_chunk, ops)
        if verbose:
            print("registers used:", n_regs)

        nc.sync.dma_start(tau_d[:, :].rearrange("(p f) d -> p (f d)", p=P),
                          out_chunk[:, :])
    if not nc.is_finalized():
        nc.finalize()
    return nc


def kernel(**inputs):
    q = np.ascontiguousarray(inputs["q"], np.float32)
    qd = np.ascontiguousarray(inputs["qd"], np.float32)
    qdd = np.ascontiguousarray(inputs["qdd_des"], np.float32)
    C = host_consts(inputs["rot_fix"], inputs["trans_fix"], inputs["joint_axes"],
                    inputs["mass"], inputs["com"], inputs["inertia"],
                    inputs["damping"])
    nc = _build_nc(C)

    from concourse.bass_utils import run_bass_kernel_spmd

    in_maps = []
    for i in range(N_CORES):
        sl = slice(i * SHARD, (i + 1) * SHARD)
        in_maps.append({"q": q[sl], "qd": qd[sl], "qdd": qdd[sl]})
    res = run_bass_kernel_spmd(nc, in_maps, list(range(N_CORES)))
    out = np.concatenate([res.results[i]["tau"] for i in range(N_CORES)], 0)
    return out.astype(np.float32)


# revision 30
# speedup vs baseline: 104.4102x; 104.4102x over previous
"""Trainium2 Bass kernel: batched recursive Newton-Euler inverse dynamics
(7-dof serial chain) — data-parallel over 8 NeuronCores.

Per core, the 65536-row shard lives as fp32 planes [128 part, 512 free].
Per-link parameters are baked in as immediate constants. The physics is
emitted through a symbolic layer (Val = a*plane + c) that prunes zeros,
folds scales, and chains every n-term linear combination into n-1 fused
scalar_tensor_tensor ops. Ops are recorded into a tiny IR, dead code is
eliminated, and emission assigns temporaries to SBUF "registers" via
linear-scan liveness (Tile pool slot rotation is strict round-robin, so
naive tmp pools deadlock).
"""

import math
from contextlib import ExitStack

import numpy as np

P = 128
D = 7
N_CORES = 8
BATCH = 524288
SHARD = BATCH // N_CORES      # 65536
FD = SHARD // P               # 512


# ---------------------------------------------------------------------------
# symbolic value: a * plane + c   (plane None -> pure constant)
# ---------------------------------------------------------------------------
class Val:
    __slots__ = ("pl", "a", "c")

    def __init__(self, pl, a=1.0, c=0.0):
        self.pl = pl
        self.a = float(a)
        self.c = float(c)
        if pl is None:
            self.a = 0.0

    @property
    def is_const(self):
        return self.pl is None or self.a == 0.0


def VC(c):
    return Val(None, 0.0, c)


class Builder:
    """Backend-agnostic emitter. Each primitive is exactly one instruction."""

    def __init__(self):
        self.n_2src = 0
        self.n_1src = 0
        self.n_trig = 0
        self.phase = ""
        self._ones = None

    # ---- primitives (backends) ----
    def p_stt(self, in0, scalar, in1, op1, dest=None):
        raise NotImplementedError

    def p_tt(self, in0, in1, op, dest=None):
        raise NotImplementedError

    def p_affine(self, in0, scale, bias, dest=None):
        raise NotImplementedError

    def p_sin(self, in0, scale, bias):
        raise NotImplementedError

    def p_ones(self):
        raise NotImplementedError

    def inp(self, name, j):
        raise NotImplementedError

    def out_ap(self, j):
        raise NotImplementedError

    def f_ap(self, j, i):
        raise NotImplementedError

    def state_ap(self, j, i):
        raise NotImplementedError

    def plane_key(self, pl):
        return id(pl)

    def same_plane(self, a, b):
        return a is b

    # ---- helpers ----
    def ones(self):
        if self._ones is None:
            self._ones = self.p_ones()
        return self._ones

    def sincos(self, j):
        s = Val(self.p_sin(self.inp("q", j), 1.0, 0.0))
        c = Val(self.p_sin(self.inp("q", j), 1.0, math.pi / 2))
        self.n_trig += 2
        return s, c

    def lin(self, vals, coefs, const=0.0, dest=None, exact=False, scale_free=False):
        terms = {}
        c_acc = float(const)
        for v, k in zip(vals, coefs):
            k = float(k)
            if k == 0.0:
                continue
            c_acc += k * v.c
            if v.pl is not None and v.a != 0.0:
                key = self.plane_key(v.pl)
                if key in terms:
                    terms[key][1] += k * v.a
                else:
                    terms[key] = [v.pl, k * v.a]
        tl = [(pl, k) for pl, k in terms.values() if k != 0.0]
        if not tl:
            if dest is not None:
                self.n_1src += 1
                self.p_affine(self.ones(), c_acc, 0.0, dest=dest)
                return Val(dest, 1.0, 0.0)
            return VC(c_acc)
        if c_acc != 0.0:
            tl.append((self.ones(), c_acc))
        if len(tl) == 1:
            pl, k = tl[0]
            if dest is not None:
                self.n_1src += 1
                self.p_affine(pl, k, 0.0, dest=dest)
                return Val(dest, 1.0, 0.0)
            if exact and k != 1.0:
                self.n_1src += 1
                return Val(self.p_affine(pl, k, 0.0), 1.0, 0.0)
            return Val(pl, k, 0.0)
        tl.sort(key=lambda t: abs(t[1]))
        cur_pl, cur_k = tl[0]
        for i in range(1, len(tl)):
            pl_i, k_i = tl[i]
            is_last = i == len(tl) - 1
            use_dest = dest is not None and is_last and (scale_free or k_i == 1.0)
            d = dest if use_dest else None
            self.n_2src += 1
            cur_pl = self.p_stt(cur_pl, cur_k / k_i, pl_i, "add", dest=d)
            cur_k = k_i
        if dest is not None and not self.same_plane(cur_pl, dest):
            self.n_1src += 1
            self.p_affine(cur_pl, cur_k, 0.0, dest=dest)
            return Val(dest, 1.0, 0.0)
        if dest is not None:
            return Val(dest, cur_k if scale_free else 1.0, 0.0)
        if exact and cur_k != 1.0:
            self.n_1src += 1
            return Val(self.p_affine(cur_pl, cur_k, 0.0), 1.0, 0.0)
        return Val(cur_pl, cur_k, 0.0)

    def mov(self, v, dest):
        self.n_1src += 1
        if v.pl is None:
            self.p_affine(self.ones(), v.c, 0.0, dest=dest)
        else:
            self.p_affine(v.pl, v.a, v.c, dest=dest)
        return Val(dest, 1.0, 0.0)

    def mul(self, x, y):
        if x.is_const and y.is_const:
            return VC(x.c * y.c)
        if x.is_const:
            x, y = y, x
        if y.is_const:
            return Val(x.pl, x.a * y.c, x.c * y.c)
        xp, yp = x, y
        if xp.c != 0.0:
            self.n_1src += 1
            xp = Val(self.p_affine(xp.pl, 1.0, xp.c / xp.a), xp.a, 0.0)
        if yp.c != 0.0:
            self.n_1src += 1
            yp = Val(self.p_affine(yp.pl, 1.0, yp.c / yp.a), yp.a, 0.0)
        self.n_2src += 1
        out = self.p_tt(xp.pl, yp.pl, "mult")
        return Val(out, xp.a * yp.a, 0.0)

    def cross(self, u, v):
        out = []
        for i in range(3):
            b, c = (i + 1) % 3, (i + 2) % 3
            m1 = self.mul(u[b], v[c])
            m2 = self.mul(u[c], v[b])
            out.append((m1, m2))
        return out

    def matvec(self, M, v):
        return [self.lin(v, [M[i][0], M[i][1], M[i][2]]) for i in range(3)]

    def givens(self, c, s, k, sgn, w, inverse, dests=None):
        a, b = (k + 1) % 3, (k + 2) % 3
        sg = -sgn if inverse else sgn
        out = [None, None, None]
        if w[a].is_const and w[b].is_const:
            out[a] = self.lin([c, s], [w[a].c, -sg * w[b].c])
            out[b] = self.lin([s, c], [sg * w[a].c, w[b].c])
        else:
            ca = self.mul(c, w[a])
            cb = self.mul(c, w[b])
            sa = self.mul(s, w[a])
            sb = self.mul(s, w[b])
            da = dests[a] if dests else None
            db = dests[b] if dests else None
            out[a] = self.lin([ca, sb], [1.0, -sg], dest=da, scale_free=True)
            out[b] = self.lin([sa, cb], [sg, 1.0], dest=db, scale_free=True)
        out[k] = w[k]
        if dests:
            if dests[a] is not None and (out[a].pl is None
                                         or not self.same_plane(out[a].pl, dests[a])):
                out[a] = self.mov(out[a], dests[a])
            if dests[b] is not None and (out[b].pl is None
                                         or not self.same_plane(out[b].pl, dests[b])):
                out[b] = self.mov(out[b], dests[b])
            if dests[k] is not None and not w[k].is_const:
                out[k] = self.mov(w[k], dests[k])
        return out


# ---------------------------------------------------------------------------
# host-side constants
# ---------------------------------------------------------------------------
def host_consts(rot_fix, trans_fix, joint_axes, mass, com, inertia, damping):
    rot_fix = np.asarray(rot_fix, np.float64)
    trans_fix = np.asarray(trans_fix, np.float64)
    joint_axes = np.asarray(joint_axes, np.float64)
    mass = np.asarray(mass, np.float64)
    com = np.asarray(com, np.float64)
    inertia = np.asarray(inertia, np.float64)
    damping = np.asarray(damping, np.float64)
    C = {}
    C["F"] = [rot_fix[j + 1] for j in range(D)]
    C["p"] = [trans_fix[j + 1] for j in range(D)]
    ax = []
    for j in range(D):
        k = int(np.argmax(np.abs(joint_axes[j])))
        ax.append((k, float(np.sign(joint_axes[j][k]))))
    C["ax"] = ax
    C["m"] = [float(mass[j + 1]) for j in range(D)]
    C["mc"] = [mass[j + 1] * com[j + 1] for j in range(D)]
    Io = []
    for j in range(D):
        cc = com[j + 1]
        cs = np.array([[0, -cc[2], cc[1]], [cc[2], 0, -cc[0]], [-cc[1], cc[0], 0]])
        Io.append(inertia[j + 1] + mass[j + 1] * (cs @ cs.T))
    C["Io"] = Io
    C["damping"] = [float(damping[j]) for j in range(D)]
    C["G"] = 9.81
    return C


# ---------------------------------------------------------------------------
# the physics graph (backend-independent)
# ---------------------------------------------------------------------------
def build_rnea(b: Builder, C):
    Z = VC(0.0)
    vl = [Z, Z, Z]
    va = [Z, Z, Z]
    al = [Z, Z, VC(C["G"])]
    aa = [Z, Z, Z]
    fstore = [[None] * 6 for _ in range(D)]
    for j in range(D):
        F = C["F"][j]
        p = C["p"][j]
        k, sg = C["ax"][j]
        a_, b_ = (k + 1) % 3, (k + 2) % 3
        b.phase = f"fwd{j}"
        s, c = b.sincos(j)
        qd = Val(b.inp("qd", j))
        qdd = Val(b.inp("qdd", j))
        Ft = F.T.tolist()

        def dvec(x, y):
            out = []
            for i in range(3):
                bb, cc = (i + 1) % 3, (i + 2) % 3
                out.append(b.lin([x[i], y[cc], y[bb]], [1.0, -p[bb], p[cc]]))
            return out

        u_vl = b.matvec(Ft, dvec(vl, va))
        u_va = b.matvec(Ft, va)
        u_al = b.matvec(Ft, dvec(al, aa))
        u_aa = b.matvec(Ft, aa)
        std = lambda i: b.state_ap(j, i)
        vl_i = b.givens(c, s, k, sg, u_vl, True, dests=[std(0), std(1), std(2)])
        va_r = b.givens(c, s, k, sg, u_va, True,
                        dests=[std(3 + i) if i != k else None for i in range(3)])
        va_i = list(va_r)
        va_i[k] = b.lin([va_r[k], qd], [1.0, sg], dest=std(3 + k), scale_free=True)
        al_r = b.givens(c, s, k, sg, u_al, True,
                        dests=[std(6 + i) if i == k else None for i in range(3)])
        aa_r = b.givens(c, s, k, sg, u_aa, True)
        aa_i = list(aa_r)
        aa_i[k] = b.lin([aa_r[k], qdd], [1.0, sg], dest=std(9 + k), scale_free=True)
        ek = [0.0, 0.0, 0.0]
        ek[k] = 1.0
        al_i = list(al_r)
        for i in (a_, b_):
            bb, cc = (i + 1) % 3, (i + 2) % 3
            cva = b.lin([va_i[bb], va_i[cc]], [ek[cc], -ek[bb]])
            m1 = b.mul(cva, qd)
            aa_i[i] = b.lin([aa_r[i], m1], [1.0, sg], dest=std(9 + i),
                            scale_free=True)
            cvl = b.lin([vl_i[bb], vl_i[cc]], [ek[cc], -ek[bb]])
            m2 = b.mul(cvl, qd)
            al_i[i] = b.lin([al_r[i], m2], [1.0, sg], dest=std(6 + i),
                            scale_free=True)
        vl, va, al, aa = vl_i, va_i, al_i, aa_i

        # ---- force for this joint ----
        b.phase = f"force{j}"
        m = C["m"][j]
        mc = C["mc"][j].tolist()
        Io = C["Io"][j]
        Iv_l = [b.lin([vl[i], va[(i + 1) % 3], va[(i + 2) % 3]],
                      [m, mc[(i + 2) % 3], -mc[(i + 1) % 3]]) for i in range(3)]
        Ia_l = [b.lin([al[i], aa[(i + 1) % 3], aa[(i + 2) % 3]],
                      [m, mc[(i + 2) % 3], -mc[(i + 1) % 3]]) for i in range(3)]
        Iv_a = [b.lin([va[0], va[1], va[2], vl[(i + 2) % 3], vl[(i + 1) % 3]],
                      [Io[i][0], Io[i][1], Io[i][2],
                       mc[(i + 1) % 3], -mc[(i + 2) % 3]]) for i in range(3)]
        Ia_a = [b.lin([aa[0], aa[1], aa[2], al[(i + 2) % 3], al[(i + 1) % 3]],
                      [Io[i][0], Io[i][1], Io[i][2],
                       mc[(i + 1) % 3], -mc[(i + 2) % 3]]) for i in range(3)]
        cv1 = b.cross(va, Iv_l)
        for i in range(3):
            m1, m2 = cv1[i]
            fstore[j][i] = b.lin([Ia_l[i], m1, m2], [1.0, 1.0, -1.0],
                                 dest=b.f_ap(j, i), scale_free=True)
        cv2 = b.cross(va, Iv_a)
        cv3 = b.cross(vl, Iv_l)
        for i in range(3):
            m1, m2 = cv2[i]
            m3, m4 = cv3[i]
            fstore[j][3 + i] = b.lin([Ia_a[i], m1, m2, m3, m4],
                                     [1.0, 1.0, -1.0, 1.0, -1.0],
                                     dest=b.f_ap(j, 3 + i), scale_free=True)

    # ---- backward pass ----
    cl = [Z, Z, Z]
    ca = [Z, Z, Z]
    for j in range(D - 1, -1, -1):
        F = C["F"][j]
        p = C["p"][j]
        k, sg = C["ax"][j]
        pp = (F.T @ p).tolist()
        b.phase = f"bwd{j}"
        s, c = b.sincos(j)
        f_l = fstore[j][:3]
        f_a = fstore[j][3:]
        tl = [b.lin([f_l[i], cl[i]], [1.0, 1.0]) for i in range(3)]
        ta = [b.lin([f_a[i], ca[i]], [1.0, 1.0]) for i in range(3)]
        b.lin([ta[k], Val(b.inp("qd", j))], [sg, C["damping"][j]],
              dest=b.out_ap(j))
        if j == 0:
            continue
        w_l = b.givens(c, s, k, sg, tl, False)
        w_a = b.givens(c, s, k, sg, ta, False)
        x = []
        for i in range(3):
            bb, cc = (i + 1) % 3, (i + 2) % 3
            x.append(b.lin([w_a[i], w_l[cc], w_l[bb]], [1.0, pp[bb], -pp[cc]]))
        cl = b.matvec(F.tolist(), w_l)
        ca = b.matvec(F.tolist(), x)


# ---------------------------------------------------------------------------
# numpy backend (validation)
# ---------------------------------------------------------------------------
class NumpyBuilder(Builder):
    def __init__(self, q, qd, qdd):
        super().__init__()
        self.q, self.qd, self.qdd = q, qd, qdd
        self.N = q.shape[0]
        self.out = np.zeros((self.N, D), np.float32)
        self._f = {}

    def _w(self, r, dest):
        if dest is not None:
            dest[...] = r
            return dest
        return r

    def _f32(self, x):
        return np.asarray(x, np.float32)

    def p_stt(self, in0, scalar, in1, op1, dest=None):
        r = self._f32(in0 * np.float32(scalar))
        if op1 == "add":
            r = self._f32(r + in1)
        elif op1 == "subtract":
            r = self._f32(r - in1)
        else:
            r = self._f32(r * in1)
        return self._w(r, dest)

    def p_tt(self, in0, in1, op, dest=None):
        if op == "mult":
            r = self._f32(in0 * in1)
        elif op == "add":
            r = self._f32(in0 + in1)
        else:
            r = self._f32(in0 - in1)
        return self._w(r, dest)

    def p_affine(self, in0, scale, bias, dest=None):
        return self._w(self._f32(in0 * np.float32(scale) + np.float32(bias)), dest)

    def p_sin(self, in0, scale, bias):
        return self._f32(np.sin(self._f32(in0 * np.float32(scale) + np.float32(bias))))

    def p_ones(self):
        return np.ones(self.N, np.float32)

    def inp(self, name, j):
        return {"q": self.q, "qd": self.qd, "qdd": self.qdd}[name][:, j].astype(
            np.float32
        )

    def out_ap(self, j):
        return self.out[:, j]

    def f_ap(self, j, i):
        key = (j, i)
        if key not in self._f:
            self._f[key] = np.empty(self.N, np.float32)
        return self._f[key]

    def state_ap(self, j, i):
        return np.empty(self.N, np.float32)


def rnea_numpy(q, qd, qdd, rot_fix, trans_fix, joint_axes, mass, com, inertia,
               damping):
    C = host_consts(rot_fix, trans_fix, joint_axes, mass, com, inertia, damping)
    b = NumpyBuilder(q, qd, qdd)
    build_rnea(b, C)
    return b.out


# ---------------------------------------------------------------------------
# IR backend: records ops on integer-token planes
# ---------------------------------------------------------------------------
class IRBuilder(Builder):
    def __init__(self):
        super().__init__()
        self.ops = []   # (kind, out_token, in_tokens, params, phase)
        self._n = 0
        self.phase = ""

    def _tmp(self):
        self._n += 1
        return ("t", self._n)

    def plane_key(self, pl):
        return pl

    def same_plane(self, a, b):
        return a == b

    def p_stt(self, in0, scalar, in1, op1, dest=None):
        out = dest if dest is not None else self._tmp()
        self.ops.append(("stt", out, (in0, in1), (float(scalar), op1),
                         self.phase))
        return out

    def p_tt(self, in0, in1, op, dest=None):
        out = dest if dest is not None else self._tmp()
        self.ops.append(("tt", out, (in0, in1), (op,), self.phase))
        return out

    def p_affine(self, in0, scale, bias, dest=None):
        out = dest if dest is not None else self._tmp()
        self.ops.append(("affine", out, (in0,), (float(scale), float(bias)),
                         self.phase))
        return out

    def p_sin(self, in0, scale, bias):
        out = self._tmp()
        self.ops.append(("sin", out, (in0,), (float(scale), float(bias)),
                         self.phase))
        return out

    def p_ones(self):
        out = ("ones",)
        self.ops.append(("memset", out, (), (1.0,), self.phase))
        return out

    def inp(self, name, j):
        return ("in", name, j)

    def out_ap(self, j):
        return ("out", j)

    def f_ap(self, j, i):
        return ("f", j, i)

    def state_ap(self, j, i):
        return self._tmp()


def dce(ops):
    """drop ops whose results are never used (named 'out'/'f' sinks are live;
    'f' only if read)."""
    needed = set()
    keep = [False] * len(ops)
    for idx in range(len(ops) - 1, -1, -1):
        kind, out, ins, params, phase = ops[idx]
        if out[0] == "out" or out in needed:
            keep[idx] = True
            for t in ins:
                needed.add(t)
    return [op for k2, op in zip(keep, ops) if k2]


def ir_stats(ops):
    from collections import Counter

    c = Counter(k for k, *_ in ops)
    # liveness: peak concurrent tmp planes
    last_use = {}
    for idx, (kind, out, ins, params, phase) in enumerate(ops):
        for t in ins:
            if t[0] == "t":
                last_use[t] = idx
    live = set()
    peak = 0
    for idx, (kind, out, ins, params, phase) in enumerate(ops):
        if out[0] == "t":
            live.add(out)
        peak = max(peak, len(live))
        for t in ins:
            if t[0] == "t" and last_use.get(t) == idx:
                live.discard(t)
    return dict(c), peak


def interleave(ops, window=8):
    """Topological reorder that avoids scheduling an op directly after the op
    that produced one of its inputs (the DVE pays an SBUF read-after-write
    bubble between dependent back-to-back instructions). Picks among the
    first `window` ready ops in original order."""
    n = len(ops)
    prod = {}
    for i, (kind, out, ins, params, phase) in enumerate(ops):
        prod[out] = i
    succs = [[] for _ in range(n)]
    ndeps = [0] * n
    for i, (kind, out, ins, params, phase) in enumerate(ops):
        seen = set()
        for t in ins:
            j = prod.get(t)
            if j is not None and j not in seen:
                seen.add(j)
                succs[j].append(i)
                ndeps[i] += 1
        # keep multiple writers of the same named plane ordered (shouldn't
        # happen except memset/out rewrites in bench mode)
    import heapq

    ready = [i for i in range(n) if ndeps[i] == 0]
    heapq.heapify(ready)
    order = []
    last_out = None
    while ready:
        cand = heapq.nsmallest(window, ready)
        pick = None
        for i in cand:
            ins_i = ops[i][2]
            if last_out is None or last_out not in ins_i:
                pick = i
                break
        if pick is None:
            pick = cand[0]
        ready.remove(pick)
        heapq.heapify(ready)
        order.append(pick)
        last_out = ops[pick][1]
        for s in succs[pick]:
            ndeps[s] -= 1
            if ndeps[s] == 0:
                heapq.heappush(ready, s)
    assert len(order) == n
    return [ops[i] for i in order]


def build_ir(C, reorder=False):
    b = IRBuilder()
    build_rnea(b, C)
    ops = dce(b.ops)
    if reorder:
        ops = interleave(ops)
    return ops, b


# ---------------------------------------------------------------------------
# bass emission from IR
# ---------------------------------------------------------------------------
def emit_bass(nc, tc, pools, chunks, out_chunk, ops, fd=FD, bench_alias_out=False,
              gpsimd_sel=None, dtype16=False):
    from concourse import mybir

    f32 = mybir.dt.float32
    fdt = mybir.dt.float16 if dtype16 else mybir.dt.float32
    ALU = {"add": mybir.AluOpType.add, "subtract": mybir.AluOpType.subtract,
           "mult": mybir.AluOpType.mult}

    last_use = {}
    for idx, (kind, out, ins, params, phase) in enumerate(ops):
        for t in ins:
            if t[0] == "t":
                last_use[t] = idx

    ftiles = {}
    tmp_ap = {}         # token -> AP
    reg_of = {}         # token -> reg index
    free_regs = []
    n_regs = 0
    serial = 0

    def named_ap(tok):
        nonlocal serial
        if tok[0] == "in":
            _, name, j = tok
            v = chunks[name].rearrange("p (f d) -> p d f", d=D)
            return v[:, j, :]
        if tok[0] == "out":
            base = chunks["qdd"] if bench_alias_out else out_chunk
            v = base.rearrange("p (f d) -> p d f", d=D)
            return v[:, tok[1], :]
        if tok[0] == "f":
            _, j, i = tok
            if j not in ftiles:
                serial += 1
                ftiles[j] = pools["fst"].tile([P, 6 * fd], fdt, tag=f"f{j}",
                                              name=f"f{j}", bufs=1)
            t = ftiles[j]
            return t[:, i * fd:(i + 1) * fd]
        if tok[0] == "ones":
            return ones_ap
        raise KeyError(tok)

    def get_ap(tok):
        if tok[0] == "t":
            return tmp_ap[tok]
        return named_ap(tok)

    def alloc_out(tok, idx):
        nonlocal n_regs, serial
        if tok[0] != "t":
            return named_ap(tok)
        if free_regs:
            r = free_regs.pop()
        else:
            r = n_regs
            n_regs += 1
        reg_of[tok] = r
        serial += 1
        t = pools["reg"].tile([P, fd], fdt, tag=f"r{r}", name=f"v{serial}",
                              bufs=1)
        tmp_ap[tok] = t[:, :]
        return tmp_ap[tok]

    def release_ins(ins, idx):
        for t in ins:
            if t[0] == "t" and last_use.get(t) == idx:
                r = reg_of.pop(t, None)
                if r is not None:
                    free_regs.append(r)

    ones_ap = None
    n_gp = 0
    for idx, (kind, out, ins, params, phase) in enumerate(ops):
        if kind == "memset":
            serial += 1
            t = pools["misc"].tile([P, fd], fdt, tag="ones", name="ones", bufs=1)
            ones_ap = t[:, :]
            nc.vector.memset(ones_ap, 1.0)
            continue
        out_ap = alloc_out(out, idx)
        # GPSIMD (Pool) supports tensor_tensor but not scalar_tensor_tensor
        use_gp = gpsimd_sel is not None and kind == "tt" and \
            gpsimd_sel(idx, kind, phase)
        eng = nc.gpsimd if use_gp else nc.vector
        if use_gp:
            n_gp += 1
        if kind == "stt":
            scalar, op1 = params
            eng.scalar_tensor_tensor(out_ap, get_ap(ins[0]), scalar,
                                     get_ap(ins[1]),
                                     mybir.AluOpType.mult, ALU[op1])
        elif kind == "tt":
            eng.tensor_tensor(out_ap, get_ap(ins[0]), get_ap(ins[1]),
                              ALU[params[0]])
        elif kind == "affine":
            scale, bias = params
            nc.scalar.activation(out_ap, get_ap(ins[0]),
                                 mybir.ActivationFunctionType.Copy,
                                 bias=bias, scale=scale)
        elif kind == "sin":
            # sin(x + b) with range reduction: the ACT Sin spline is only
            # accurate for |arg| <= pi. r = round((x+b)/2pi) via the fp32
            # magic-constant trick; z = x - 2pi*r; result = Sin(z + b).
            scale, bias = params
            assert scale == 1.0
            src = get_ap(ins[0])
            TWO_PI = 2.0 * math.pi
            MAGIC = 12582912.0  # 1.5 * 2**23
            serial += 1
            scr = pools["misc"].tile([P, fd], f32, tag="trigscratch",
                                     name=f"trs{serial}", bufs=2)
            serial += 1
            zscr = pools["misc"].tile([P, fd], f32, tag="trigz",
                                      name=f"trz{serial}", bufs=2)
            u = scr[:, :]
            z = zscr[:, :]
            Copy = mybir.ActivationFunctionType.Copy
            nc.scalar.activation(u, src, Copy, bias=bias / TWO_PI,
                                 scale=1.0 / TWO_PI)
            nc.scalar.activation(u, u, Copy, bias=MAGIC, scale=1.0)
            nc.scalar.activation(u, u, Copy, bias=-MAGIC, scale=1.0)
            nc.vector.scalar_tensor_tensor(z, u, -TWO_PI, src,
                                           mybir.AluOpType.mult,
                                           mybir.AluOpType.add)
            nc.scalar.activation(out_ap, z,
                                 mybir.ActivationFunctionType.Sin,
                                 bias=bias, scale=1.0)
        else:
            raise ValueError(kind)
        release_ins(ins, idx)
    return n_regs, n_gp


def _build_nc(C, verbose=False, repeat=1, gpsimd_sel=None, dtype16=False,
              reorder=False):
    import concourse.bacc as bacc
    import concourse.tile as tile_mod
    from concourse import mybir

    ops, bstat = build_ir(C, reorder=reorder)
    if verbose:
        stats, peak = ir_stats(ops)
        print("IR ops:", stats, "peak live tmps:", peak)

    nc = bacc.Bacc()
    f32 = mybir.dt.float32
    # register pi/2 as a const AP (Sin activation bias must be a const AP)
    halfpi = float(math.pi / 2)
    _ct = nc.alloc_sbuf_tensor("const-f32-halfpi", [128, 1], f32)
    nc.gpsimd.memset(_ct.ap(), halfpi)
    nc.const_aps.aps[(f32, halfpi)] = _ct.ap()
    nc.all_engine_barrier()
    q_d = nc.dram_tensor("q", [SHARD, D], f32, kind="ExternalInput")
    qd_d = nc.dram_tensor("qd", [SHARD, D], f32, kind="ExternalInput")
    qdd_d = nc.dram_tensor("qdd", [SHARD, D], f32, kind="ExternalInput")
    tau_d = nc.dram_tensor("tau", [SHARD, D], f32, kind="ExternalOutput")

    with ExitStack() as ctx:
        tc = ctx.enter_context(tile_mod.TileContext(nc))
        io_pool = ctx.enter_context(tc.tile_pool(name="io", bufs=1))
        fst_pool = ctx.enter_context(tc.tile_pool(name="fst", bufs=1))
        reg_pool = ctx.enter_context(tc.tile_pool(name="reg", bufs=1))
        misc_pool = ctx.enter_context(tc.tile_pool(name="misc", bufs=1))
        pools = {"io": io_pool, "fst": fst_pool, "reg": reg_pool,
                 "misc": misc_pool}

        chunks = {}
        for name, dram in (("q", q_d), ("qd", qd_d), ("qdd", qdd_d)):
            t = io_pool.tile([P, D * FD], f32, tag=f"io_{name}",
                             name=f"ch_{name}", bufs=1)
            nc.sync.dma_start(t[:, :],
                              dram[:, :].rearrange("(p f) d -> p (f d)", p=P))
            chunks[name] = t

        if repeat == 1:
            # out chunk shares the qdd slot (qdd is fully consumed by the
            # forward pass before any tau is written)
            out_chunk = io_pool.tile([P, D * FD], f32, tag="io_qdd",
                                     name="ch_out", bufs=1)
            n_regs, n_gp = emit_bass(nc, tc, pools, chunks, out_chunk, ops,
                                     gpsimd_sel=gpsimd_sel, dtype16=dtype16)
        else:
            # bench mode: tau lands in the qdd chunk itself (timing only)
            out_chunk = chunks["qdd"]
            for _ in range(repeat):
                n_regs, n_gp = emit_bass(nc, tc, pools, chunks, out_chunk, ops,
                                         bench_alias_out=True,
                                         gpsimd_sel=gpsimd_sel,
                                         dtype16=dtype16)
        if verbose:
            print("registers used:", n_regs, "gpsimd ops:", n_gp)

        nc.sync.dma_start(tau_d[:, :].rearrange("(p f) d -> p (f d)", p=P),
                          out_chunk[:, :])
    if not nc.is_finalized():
        nc.finalize()
    return nc


def kernel(**inputs):
    q = np.ascontiguousarray(inputs["q"], np.float32)
    qd = np.ascontiguousarray(inputs["qd"], np.float32)
    qdd = np.ascontiguousarray(inputs["qdd_des"], np.float32)
    C = host_consts(inputs["rot_fix"], inputs["trans_fix"], inputs["joint_axes"],
                    inputs["mass"], inputs["com"], inputs["inertia"],
                    inputs["damping"])
    nc = _build_nc(C)

    from concourse.bass_utils import run_bass_kernel_spmd

    in_maps = []
    for i in range(N_CORES):
        sl = slice(i * SHARD, (i + 1) * SHARD)
        in_maps.append({"q": q[sl], "qd": qd[sl], "qdd": qdd[sl]})
    res = run_bass_kernel_spmd(nc, in_maps, list(range(N_CORES)))
    out = np.concatenate([res.results[i]["tau"] for i in range(N_CORES)], 0)
    return out.astype(np.float32)
